# revision 1
# baseline (speedup 1.0000x reference)
"""Trainium2 Bass kernel for fused attention block (QKV proj + RoPE + SDPA + out proj).

Reference computation (B=4, S=2048, HID=2048, H=16, D=128, fp32):
    qkv = hidden @ w_qkv; q,k,v split per head
    q,k = RoPE(q,k, cos,sin)
    attn = softmax(q k^T / sqrt(D)) v          (per batch, head)
    out  = attn.reshape(B,S,H*D) @ w_o

Sharding (8 cores): core c -> (batch b=c//2, head-group g=c%2 of 8 heads).
Each core computes a partial output [S, HID] over its 8 heads; the host sums
the two head-group partials per batch.

Per-core kernel (all matmuls fp32r = fp22-rounded fp32, 1 cyc/row at N>=256):
  Phase 1: QKV projection from X^T (host-pretransposed hidden), with RoPE
           fused at PSUM eviction. rotate_half is a partition rotation by 64
           done with two SBUF->SBUF DMAs; the sign and the 1/sqrt(D) scale are
           folded into host-prepared cos/sin tensors. Q^T,K^T stored [D,S]
           per head (DRAM scratch), V stored [S, 8*128] natural.
  Phase 2: per head: scores^T chunk [sk,128 x sq,512] = K_chunk @ Q^T_block
           (single 128-contraction matmul), Exp on ACT -> P~^T; denominators
           via ones[128,128] matmul (colsum broadcast to all partitions);
           O^T += V_chunk^T @ P~^T accumulated over sk; normalize by 1/d at
           PSUM eviction. O^T kept in SBUF [128, 8, 2048].
  Phase 3: out = O_flat @ w_o via lhsT = O^T slices, rhs = w_o (head-tiled).

Softmax is computed without max subtraction: scores here are O(1)-scaled
(|s| < ~15 even at 5+ sigma), so exp() is well within fp32 range.
"""

import os
import sys
import types

sys.path.insert(0, "/opt/trn_rl_repo")

import numpy as np

B, S, HID = 4, 2048, 2048
H, D = 16, 128
HG = 8            # heads per core (head-group)
NCORES = 8
SB = 512          # s-block (matmul free dim)
NSB = S // SB     # 4
NKT = HID // 128  # 16 k-tiles over hidden
NSK = S // 128    # 16 key chunks

_STATE = {}
LAST_RESULTS = None


def _ensure_ntff_hook():
    """bass_utils wants antenv.axon_hooks for NTFF tracing under axon; this
    container's antenv lacks it. Register the ctypes-backed hook."""
    try:
        from antenv import axon_hooks  # noqa: F401
        return
    except ImportError:
        pass
    import antenv
    from trn_agent_boot.trn_boot import _ntff_profile_via_ctypes

    mod = types.ModuleType("antenv.axon_hooks")
    _hook = [None]
    mod.set_axon_ntff_profile_hook = lambda h: _hook.__setitem__(0, h)
    mod.get_axon_ntff_profile_hook = lambda: _hook[0]
    sys.modules["antenv.axon_hooks"] = mod
    antenv.axon_hooks = mod
    mod.set_axon_ntff_profile_hook(
        _ntff_profile_via_ctypes("/opt/axon/libaxon_pjrt.so")
    )


def _build():
    import concourse.mybir as mybir
    import concourse.tile as tile
    from concourse import bacc

    F32 = mybir.dt.float32
    F32R = mybir.dt.float32r
    BF16 = mybir.dt.bfloat16
    EXP = mybir.ActivationFunctionType.Exp

    nc = bacc.Bacc(None, target_bir_lowering=False, debug=False)

    x_t = nc.dram_tensor("x_t", [HID, S], F32R, kind="ExternalInput")
    w_q = nc.dram_tensor("w_q", [128, NKT, HG * 128], F32R, kind="ExternalInput")
    w_k = nc.dram_tensor("w_k", [128, NKT, HG * 128], F32R, kind="ExternalInput")
    w_v = nc.dram_tensor("w_v", [128, NKT, HG * 128], F32R, kind="ExternalInput")
    cos_q = nc.dram_tensor("cos_q", [128, S], F32R, kind="ExternalInput")
    sin_q = nc.dram_tensor("sin_q", [128, S], F32R, kind="ExternalInput")
    cos_k = nc.dram_tensor("cos_k", [128, S], F32R, kind="ExternalInput")
    sin_k = nc.dram_tensor("sin_k", [128, S], F32R, kind="ExternalInput")
    ones_in = nc.dram_tensor("ones_in", [128, 128], F32R, kind="ExternalInput")
    w_o = nc.dram_tensor("w_o", [128, HG, HID], F32R, kind="ExternalInput")
    out_p = nc.dram_tensor("out_p", [S, HID], F32, kind="ExternalOutput")

    SH = S // 2  # phase-1 processes s in halves to keep one unified pool scope

    with tile.TileContext(nc) as tc:
        with tc.tile_pool(name="dram", bufs=1, space="DRAM") as dr:
            # Per-head / per-pair scratch tensors: separate tiles give Tile
            # fine-grained cross-phase deps, so attention head h can start as
            # soon as ITS q/k/v are written, overlapping phase-2's ACT-bound
            # exp work with phase-1's PE-bound tail.
            q_ropes = [dr.tile([128, S], F32R, name=f"q_rope{c}") for c in range(HG)]
            k_ropes = [dr.tile([128, S], F32R, name=f"k_rope{c}") for c in range(HG)]
            v_pairs = [dr.tile([S, 256], F32R, name=f"v_pair{j}") for j in range(HG // 2)]
            o_t = dr.tile([HG, 128, S], F32R)

            with tc.tile_pool(name="p2h", bufs=2) as hp:
                # ---------------- Phase 1: QKV projection + RoPE ----------
                with tc.tile_pool(name="p1xt", bufs=1) as xtp:
                    NSBH = SH // SB  # s-blocks per half

                    def v_pass(sh, xt, vwp, vevp, vpsp):
                        for vc in range(4):
                            vsl = slice(vc * 256, (vc + 1) * 256)
                            wvc = vwp.tile([128, NKT, 256], F32R, tag="wv")
                            nc.sync.dma_start(wvc[:], w_v[:, :, vsl])
                            for ss in range(SH // 128):
                                ps = vpsp.tile([128, 256], F32, tag="ps_v")
                                for kt in range(NKT):
                                    nc.tensor.matmul(
                                        ps[:],
                                        xt[:, kt, ss * 128 : (ss + 1) * 128],
                                        wvc[:, kt, :],
                                        start=(kt == 0),
                                        stop=(kt == NKT - 1),
                                    )
                                vt = vevp.tile([128, 256], F32R, tag="vt")
                                nc.vector.tensor_copy(vt[:], ps[:])
                                row = sh * SH + ss * 128
                                nc.gpsimd.dma_start(
                                    v_pairs[vc][row : row + 128, :], vt[:]
                                )

                    with (
                        tc.tile_pool(name="p1w", bufs=2) as wp,
                        tc.tile_pool(name="p1vw", bufs=2) as vwp,
                        tc.tile_pool(name="p1cs", bufs=1) as csp,
                        tc.tile_pool(name="p1ev", bufs=4) as evp,
                        tc.tile_pool(name="p1vev", bufs=3) as vevp,
                        tc.tile_pool(name="p1ps", bufs=4, space="PSUM") as ps1,
                        tc.tile_pool(name="p1vps", bufs=2, space="PSUM") as vps,
                    ):
                        def qk_pass(sh, w_dram, cos_dram, sin_dram, ropes, xt,
                                    load_xt=False):
                            hsl = slice(sh * SH, (sh + 1) * SH)
                            cqf = sqf = None
                            for c in range(HG):
                                wc = wp.tile([128, NKT, 128], F32R, tag="w")
                                nc.sync.dma_start(
                                    wc[:], w_dram[:, :, c * 128 : (c + 1) * 128]
                                )
                                if cqf is None:
                                    if load_xt:
                                        for sb in range(NSBH):
                                            gs = slice(sh * SH + sb * SB,
                                                       sh * SH + (sb + 1) * SB)
                                            nc.sync.dma_start(
                                                xt[:, :, sb * SB : (sb + 1) * SB],
                                                x_t[:, gs].rearrange(
                                                    "(ko p) s -> p ko s", p=128),
                                            )
                                    cqf = csp.tile([128, SH], F32R, tag="cq")
                                    nc.sync.dma_start(cqf[:], cos_dram[:, hsl])
                                    sqf = csp.tile([128, SH], F32R, tag="sq")
                                    nc.sync.dma_start(sqf[:], sin_dram[:, hsl])
                                for sb in range(NSBH):
                                    lsl = slice(sb * SB, (sb + 1) * SB)
                                    gsl = slice(sh * SH + sb * SB,
                                                sh * SH + (sb + 1) * SB)
                                    ps = ps1.tile([128, SB], F32, tag="ps_qk")
                                    for kt in range(NKT):
                                        nc.tensor.matmul(
                                            ps[:],
                                            wc[:, kt, :],
                                            xt[:, kt, lsl],
                                            start=(kt == 0),
                                            stop=(kt == NKT - 1),
                                        )
                                    qt = evp.tile([128, SB], F32R, tag="qt")
                                    nc.scalar.copy(qt[:], ps[:])
                                    qs = evp.tile([128, SB], F32R, tag="qs")
                                    nc.gpsimd.dma_start(qs[0:64, :], qt[64:128, :])
                                    nc.gpsimd.dma_start(qs[64:128, :], qt[0:64, :])
                                    nc.vector.tensor_mul(qt[:], qt[:], cqf[:, lsl])
                                    nc.vector.tensor_mul(qs[:], qs[:], sqf[:, lsl])
                                    nc.vector.tensor_add(qt[:], qt[:], qs[:])
                                    nc.gpsimd.dma_start(ropes[c][:, gsl], qt[:])

                        # half 1: Q, K, V; half 2: Q, K (V-half2 runs in the
                        # next scope, overlapped with early attention heads)
                        xt = xtp.tile([128, NKT, SH], F32R, tag="xt")
                        qk_pass(0, w_q, cos_q, sin_q, q_ropes, xt, load_xt=True)
                        qk_pass(0, w_k, cos_k, sin_k, k_ropes, xt)
                        v_pass(0, xt, vwp, vevp, vps)
                        xt = xtp.tile([128, NKT, SH], F32R, tag="xt")
                        qk_pass(1, w_q, cos_q, sin_q, q_ropes, xt, load_xt=True)
                        qk_pass(1, w_k, cos_k, sin_k, k_ropes, xt)

                    # ------------ Phase 2 (+ V-half2 overlapped) ----------
                    with (
                        tc.tile_pool(name="p1vw2", bufs=2) as vwp2,
                        tc.tile_pool(name="p1vev2", bufs=3) as vevp2,
                        tc.tile_pool(name="p2pt", bufs=16) as ptp,
                        tc.tile_pool(name="p2r", bufs=2) as rp,
                        tc.tile_pool(name="p2ev", bufs=3) as evp2,
                        tc.tile_pool(name="p2c", bufs=1) as cp,
                        tc.tile_pool(name="p1vps2", bufs=1, space="PSUM") as vps2,
                        tc.tile_pool(name="p2ps_s", bufs=3, space="PSUM") as ps_s,
                        tc.tile_pool(name="p2ps_d", bufs=2, space="PSUM") as ps_d,
                        tc.tile_pool(name="p2ps_o", bufs=2, space="PSUM") as ps_o,
                    ):
                        ones = cp.tile([128, 128], F32R, tag="ones")
                        nc.sync.dma_start(ones[:], ones_in[:])

                        v_pass(1, xt, vwp2, vevp2, vps2)

                        for h in range(HG):
                            qT = hp.tile([128, S], F32R, tag="qT")
                            nc.sync.dma_start(qT[:], q_ropes[h])
                            kT = hp.tile([128, S], F32R, tag="kT")
                            nc.sync.dma_start(kT[:], k_ropes[h])
                            vh = hp.tile([128, NSK, 128], F32R, tag="vh")
                            nc.sync.dma_start(
                                vh[:],
                                v_pairs[h // 2][:, (h % 2) * 128 : (h % 2 + 1) * 128]
                                .rearrange("(so p) d -> p so d", p=128),
                            )
                            for sqb in range(NSB):
                                qsl = slice(sqb * SB, (sqb + 1) * SB)
                                psd = ps_d.tile([128, SB], F32, tag="ps_d")
                                pso = ps_o.tile([128, SB], F32, tag="ps_o")
                                pts = []
                                for sk in range(NSK):
                                    pss = ps_s.tile([128, SB], F32, tag="ps_s")
                                    nc.tensor.matmul(
                                        pss[:],
                                        kT[:, sk * 128 : (sk + 1) * 128],
                                        qT[:, qsl],
                                        start=True,
                                        stop=True,
                                    )
                                    pt = ptp.tile([128, SB], F32R, tag="pt")
                                    nc.scalar.activation(pt[:], pss[:], EXP)
                                    pts.append(pt)
                                for sk in range(NSK):
                                    nc.tensor.matmul(
                                        psd[:], ones[:], pts[sk][:],
                                        start=(sk == 0), stop=(sk == NSK - 1),
                                    )
                                for sk in range(NSK):
                                    nc.tensor.matmul(
                                        pso[:], vh[:, sk, :], pts[sk][:],
                                        start=(sk == 0), stop=(sk == NSK - 1),
                                    )
                                rec = rp.tile([128, SB], F32, tag="rec")
                                nc.vector.reciprocal(rec[:], psd[:])
                                ote = evp2.tile([128, SB], F32R, tag="ote")
                                nc.vector.tensor_mul(ote[:], pso[:], rec[:])
                                nc.gpsimd.dma_start(o_t[h, :, qsl], ote[:])

                # -------------- Phase 3: output projection ------------
                with (
                    tc.tile_pool(name="p3w", bufs=1) as wop,
                    tc.tile_pool(name="p3i", bufs=4) as otcp,
                    tc.tile_pool(name="p3o", bufs=4) as outp,
                    tc.tile_pool(name="p3ps", bufs=4, space="PSUM") as ps3,
                ):
                    wo = wop.tile([128, HG, HID], F32R, tag="wo")
                    nc.sync.dma_start(wo[:], w_o[:])
                    for sc in range(S // 128):
                        otc = otcp.tile([128, HG, 128], F32R, tag="otc")
                        nc.sync.dma_start(
                            otc[:],
                            o_t[:, :, sc * 128 : (sc + 1) * 128].rearrange(
                                "h p s -> p h s"
                            ),
                        )
                        for nb in range(HID // SB):
                            ps = ps3.tile([128, SB], F32, tag="ps3")
                            for h in range(HG):
                                nc.tensor.matmul(
                                    ps[:],
                                    otc[:, h, :],
                                    wo[:, h, nb * SB : (nb + 1) * SB],
                                    start=(h == 0),
                                    stop=(h == HG - 1),
                                )
                            ot = outp.tile([128, SB], F32, tag="out")
                            nc.vector.tensor_copy(ot[:], ps[:])
                            nc.gpsimd.dma_start(
                                out_p[sc * 128 : (sc + 1) * 128,
                                      nb * SB : (nb + 1) * SB],
                                ot[:],
                            )

    nc.compile()
    return nc


def _get_nc():
    if "nc" not in _STATE:
        _STATE["nc"] = _build()
    return _STATE["nc"]


def kernel(hidden_states, cos, sin, w_qkv, w_o):
    global LAST_RESULTS
    from concourse.bass_utils import run_bass_kernel_spmd

    trace = os.environ.get("KERNEL_TRACE", "") == "1"
    if trace:
        _ensure_ntff_hook()

    hidden_states = np.asarray(hidden_states, dtype=np.float32)
    cos = np.asarray(cos, dtype=np.float32)
    sin = np.asarray(sin, dtype=np.float32)
    w_qkv = np.asarray(w_qkv, dtype=np.float32)
    w_o = np.asarray(w_o, dtype=np.float32)

    cos_t = np.ascontiguousarray(cos.T)                      # [128, S]
    sin_t = np.ascontiguousarray(sin.T)
    sin_rot = np.concatenate([-sin_t[:64], sin_t[64:]], axis=0)
    scale = np.float32(1.0 / np.sqrt(D))
    cos_qh = np.ascontiguousarray(cos_t * scale)
    sin_qh = np.ascontiguousarray(sin_rot * scale)
    ones = np.ones((128, 128), np.float32)

    def ktile(w):  # [HID, N] -> [128, NKT, N]
        n = w.shape[1]
        return np.ascontiguousarray(w.reshape(NKT, 128, n).transpose(1, 0, 2))

    in_maps = []
    for c in range(NCORES):
        b, g = divmod(c, 2)
        cs = slice(g * HG * D, (g + 1) * HG * D)
        wq = ktile(w_qkv[:, 0:H * D][:, cs])
        wk = ktile(w_qkv[:, H * D:2 * H * D][:, cs])
        wv = ktile(w_qkv[:, 2 * H * D:3 * H * D][:, cs])
        wo_c = w_o[cs, :]
        wo_r = np.ascontiguousarray(
            wo_c.reshape(HG, 128, HID).transpose(1, 0, 2)
        )
        in_maps.append({
            "x_t": np.ascontiguousarray(hidden_states[b].T),
            "w_q": wq, "w_k": wk, "w_v": wv,
            "cos_q": cos_qh, "sin_q": sin_qh,
            "cos_k": cos_t, "sin_k": sin_rot,
            "ones_in": ones,
            "w_o": wo_r,
        })

    nc = _get_nc()
    res = run_bass_kernel_spmd(
        nc, in_maps, core_ids=list(range(NCORES)), trace=trace
    )
    LAST_RESULTS = res

    out = np.empty((B, S, HID), np.float32)
    for b in range(B):
        out[b] = res.results[2 * b]["out_p"] + res.results[2 * b + 1]["out_p"]
    return out

